# revision 5
# baseline (speedup 1.0000x reference)
"""Trainium2 Bass kernel for DiscreteGCNLayer.

Computation (per batch b):
    dw      = ternary_quantize(weight, s=0.01)            # [256, 256]
    support = x[b] @ dw                                   # [2048, 256]
    out[b]  = relu(adj[b] @ support + bias)               # [2048, 256]

Strategy: data-parallel over the batch dim (8 batches -> 8 NeuronCores),
weight/bias replicated.  Inputs are staged host-side in bf16 with layouts
chosen so the device kernel needs no on-chip transposes and every DMA
moves >=4KB-contiguous lines:

  xt_d   [256, 2048]            x[b]^T        (stage-1 lhsT tiles)
  adjt_d [4, 128, 16, 512]      adj[b]^T tiled as [nb, p, mc, j] with
                                adjt[nb, p, mc, j] = adj[b][nb*512+j, mc*128+p]
  out_d  [256, 2048]            out[b]^T, un-transposed on the host

Per core the kernel computes out^T = support^T-free form:
  stage 1: psum[m, o] += xt[ic, m-tile]^T @ dw[ic]    (natural layouts)
  stage 2: psum[oh, n] += support[:, mc, oh]^T-free matmul with
           lhsT = support chunk [128m, 128o], rhs = adjT [128m, 512n]
           (adj never transposed on chip; it IS the transposed operand)
  bias+relu ride the PSUM->SBUF eviction on the scalar engine: in the
  out^T orientation bias[o] is a per-partition scalar, so
  activation(Relu, bias=...) fuses it for free.

All matmuls are bf16 (1 cycle/row on the PE, rel-err ~5e-3 vs the fp32
reference, comfortably inside the 2e-2 gate).  HBM traffic per core drops
from 20.25MB (fp32) to ~10.3MB.
"""

import sys

import numpy as np

if "/opt/trn_rl_repo" not in sys.path:
    sys.path.insert(0, "/opt/trn_rl_repo")

B = 8
N = 2048
DIN = 256
DOUT = 256
P = 128
NBW = 512          # stage-2 moving-dim window (n columns per psum group)
NNB = N // NBW     # 4 n-blocks
MB = N // P        # 16 contraction chunks (stage 2)
IB = DIN // P      # 2 contraction chunks (stage 1)
QW = 4             # m-chunks per adjT quarter DMA
NQ = MB // QW      # 4 quarter DMAs per n-block
XW = 4             # xt column windows (stage-1 early start)
OH = DOUT // P     # 2 output-partition halves
SPARSITY = 0.01
WARMUP = 10        # junk matmuls covering PE ramp + input-DMA latency

_NC = None


def _build_nc():
    from contextlib import ExitStack

    import concourse.bass as bass
    import concourse.mybir as mybir
    import concourse.tile as tile
    from concourse import bacc

    F32 = mybir.dt.float32
    BF16 = mybir.dt.bfloat16
    Alu = mybir.AluOpType

    nc = bacc.Bacc()
    xt_d = nc.dram_tensor("xt", [DIN, N], BF16, kind="ExternalInput")
    adjt_d = nc.dram_tensor("adjt", [NNB, P, MB, NBW], BF16, kind="ExternalInput")
    w_d = nc.dram_tensor("weight", [DIN, DOUT], F32, kind="ExternalInput")
    b_d = nc.dram_tensor("bias", [DOUT], F32, kind="ExternalInput")
    out_d = nc.dram_tensor("out", [DOUT, N], BF16, kind="ExternalOutput")

    with tile.TileContext(nc) as tc, ExitStack() as ctx:
        singles = ctx.enter_context(tc.tile_pool(name="singles", bufs=1))
        aq_pool = ctx.enter_context(tc.tile_pool(name="aq", bufs=6))
        ot_pool = ctx.enter_context(tc.tile_pool(name="ot", bufs=4))
        psum_s1 = ctx.enter_context(tc.tile_pool(name="ps1", bufs=2, space="PSUM"))
        psum_s2 = ctx.enter_context(tc.tile_pool(name="ps2", bufs=4, space="PSUM"))

        # ---- inputs in flight ------------------------------------------
        # weight on the ACT queue: it is idle this early, and the HWDGE path
        # starts the transfer ~1us sooner than SWDGE descriptor prep would,
        # which matters because dw gates every stage-1 matmul.
        w_sb = singles.tile([P, IB, DOUT], F32)
        nc.scalar.dma_start(out=w_sb, in_=w_d[:].rearrange("(c p) o -> p c o", p=P))
        bias_sb = singles.tile([P, OH], F32)
        nc.scalar.dma_start(out=bias_sb, in_=b_d[:].rearrange("(c p) -> p c", p=P))

        # xt in XW column windows so stage 1 starts on window 0.
        xt_sb = singles.tile([P, IB, N], BF16)
        xt_r = xt_d[:].rearrange("(c p) m -> p c m", p=P)
        WN = N // XW
        for w in range(XW):
            nc.sync.dma_start(
                out=xt_sb[:, :, w * WN : (w + 1) * WN],
                in_=xt_r[:, :, w * WN : (w + 1) * WN],
            )

        # ---- PE warm-up ------------------------------------------------
        # The cost model's p-state ramp needs ~3us of continuous PE
        # activity to reach full clock; the real work is gated on the
        # xt/weight DMAs anyway, so spend the wait ramping.
        junk = singles.tile([P, NBW], BF16)
        nc.gpsimd.memset(junk, 1.0)  # Pool engine: keep DVE free for quantize
        jrelu = singles.tile([P, 8], BF16)
        nc.scalar.activation(jrelu, junk[:, 0:8], mybir.ActivationFunctionType.Relu)
        for i in range(WARMUP):
            jp = psum_s2.tile([P, NBW], F32, tag="ps2")
            nc.tensor.matmul(jp, lhsT=junk[:, 0:P], rhs=junk, start=True, stop=True)

        # ---- ternary-quantized weight: dw = ((w > s) - (w < -s)) * s ---
        dw = singles.tile([P, IB, DOUT], BF16)
        tpos = singles.tile([P, IB, DOUT], F32)
        tneg = singles.tile([P, IB, DOUT], F32)
        nc.vector.tensor_scalar(
            out=tpos, in0=w_sb, scalar1=SPARSITY, scalar2=SPARSITY,
            op0=Alu.is_gt, op1=Alu.mult,
        )
        nc.vector.tensor_scalar(
            out=tneg, in0=w_sb, scalar1=-SPARSITY, scalar2=SPARSITY,
            op0=Alu.is_lt, op1=Alu.mult,
        )
        nc.vector.tensor_sub(dw, tpos, tneg)

        # ---- stage-2 adjT prefetch stream ------------------------------
        aq_tiles = {}

        def start_aq(nb, q):
            aq = aq_pool.tile([P, QW, NBW], BF16, tag="aq", name=f"aq{nb}_{q}")
            nc.sync.dma_start(out=aq, in_=adjt_d[nb, :, q * QW : (q + 1) * QW, :])
            aq_tiles[(nb, q)] = aq

        for q in range(NQ):
            start_aq(0, q)

        # ---- stage 1: support[m-chunk][p, o] = sum_i x[.,i] dw[i, o] ---
        support = singles.tile([P, MB, DOUT], BF16)
        for mc in range(MB):
            sp = psum_s1.tile([P, DOUT], F32, tag="ps1")
            for ic in range(IB):
                nc.tensor.matmul(
                    sp,
                    lhsT=xt_sb[:, ic, mc * P : (mc + 1) * P],
                    rhs=dw[:, ic, :],
                    start=(ic == 0),
                    stop=(ic == IB - 1),
                )
            if mc % 2 == 0:
                nc.vector.tensor_copy(support[:, mc, :], sp)
            else:
                nc.scalar.copy(support[:, mc, :], sp)

        # ---- stage 2: outT[oh, nb] = relu(support^T-form @ adjT + b) ---
        for nb in range(NNB):
            po = [psum_s2.tile([P, NBW], F32, tag="ps2", name=f"po{nb}_{oh}")
                  for oh in range(OH)]
            for mc in range(MB):
                aq = aq_tiles[(nb, mc // QW)]
                rhs = aq[:, mc % QW, :]
                for oh in range(OH):
                    nc.tensor.matmul(
                        po[oh],
                        lhsT=support[:, mc, oh * P : (oh + 1) * P],
                        rhs=rhs,
                        start=(mc == 0),
                        stop=(mc == MB - 1),
                    )
                # prefetch the next n-block's quarters as ours retire
                if mc % QW == QW - 1 and nb + 1 < NNB:
                    start_aq(nb + 1, mc // QW)
            # relus back-to-back on ACT; stores ride SWDGE (Pool queue) so a
            # store's semaphore wait never blocks the next relu dispatch.
            ots = []
            for oh in range(OH):
                ot = ot_pool.tile([P, NBW], BF16, tag="ot")
                nc.scalar.activation(
                    ot, po[oh], mybir.ActivationFunctionType.Relu,
                    bias=bias_sb[:, oh : oh + 1],
                )
                ots.append(ot)
            for oh in range(OH):
                nc.gpsimd.dma_start(
                    out=out_d[oh * P : (oh + 1) * P, nb * NBW : (nb + 1) * NBW],
                    in_=ots[oh],
                )

    nc.compile()
    return nc


def _get_nc():
    global _NC
    if _NC is None:
        _NC = _build_nc()
    return _NC


def kernel(x, adj, weight, bias, _trace=False):
    import ml_dtypes
    from concourse import bass_utils

    bf16 = ml_dtypes.bfloat16
    x = np.asarray(x, dtype=np.float32)
    adj = np.asarray(adj, dtype=np.float32)
    weight = np.ascontiguousarray(np.asarray(weight, dtype=np.float32))
    bias = np.ascontiguousarray(np.asarray(bias, dtype=np.float32))

    nc = _get_nc()
    in_maps = []
    for b in range(B):
        xt = np.ascontiguousarray(x[b].T).astype(bf16)
        # adjt[nb, p, mc, j] = adj[b][nb*512 + j, mc*128 + p]
        adjt = np.ascontiguousarray(
            adj[b].reshape(NNB, NBW, MB, P).transpose(0, 3, 2, 1)
        ).astype(bf16)
        in_maps.append({"xt": xt, "adjt": adjt, "weight": weight, "bias": bias})

    res = bass_utils.run_bass_kernel_spmd(
        nc, in_maps, core_ids=list(range(B)), trace=_trace
    )
    out = np.stack(
        [np.asarray(r["out"]).astype(np.float32).T for r in res.results], axis=0
    )
    if _trace:
        return out, res
    return out


# revision 12
# speedup vs baseline: 1.0623x; 1.0623x over previous
"""Trainium2 Bass kernel for DiscreteGCNLayer.

Computation (per batch b):
    dw      = ternary_quantize(weight, s=0.01)            # [256, 256]
    support = x[b] @ dw                                   # [2048, 256]
    out[b]  = relu(adj[b] @ support + bias)               # [2048, 256]

Strategy: data-parallel over the batch dim (8 batches -> 8 NeuronCores),
weight/bias replicated.  Inputs are staged host-side in bf16 with layouts
chosen so the device kernel needs no on-chip transposes and every DMA
moves >=4KB-contiguous lines:

  xt_d   [256, 2048]            x[b]^T        (stage-1 lhsT tiles)
  adjt_d [4, 128, 16, 512]      adj[b]^T tiled as [nb, p, mc, j] with
                                adjt[nb, p, mc, j] = adj[b][nb*512+j, mc*128+p]
  out_d  [256, 2048]            out[b]^T, un-transposed on the host

Per core the kernel computes out^T = support^T-free form:
  stage 1: psum[m, o] += xt[ic, m-tile]^T @ dw[ic]    (natural layouts)
  stage 2: psum[oh, n] += support[:, mc, oh]^T-free matmul with
           lhsT = support chunk [128m, 128o], rhs = adjT [128m, 512n]
           (adj never transposed on chip; it IS the transposed operand)
  bias+relu ride the PSUM->SBUF eviction on the scalar engine: in the
  out^T orientation bias[o] is a per-partition scalar, so
  activation(Relu, bias=...) fuses it for free.

All matmuls are bf16 (1 cycle/row on the PE, rel-err ~5e-3 vs the fp32
reference, comfortably inside the 2e-2 gate).  HBM traffic per core drops
from 20.25MB (fp32) to ~10.3MB.
"""

import sys

import numpy as np

if "/opt/trn_rl_repo" not in sys.path:
    sys.path.insert(0, "/opt/trn_rl_repo")

B = 8
N = 2048
DIN = 256
DOUT = 256
P = 128
NBW = 512          # stage-2 moving-dim window (n columns per psum group)
NNB = N // NBW     # 4 n-blocks
MB = N // P        # 16 contraction chunks (stage 2)
IB = DIN // P      # 2 contraction chunks (stage 1)
QW = 4             # m-chunks per adjT quarter DMA
NQ = MB // QW      # 4 quarter DMAs per n-block
XW = 4             # xt column windows (stage-1 early start)
OH = DOUT // P     # 2 output-partition halves
SPARSITY = 0.01
WARMUP = 10        # junk matmuls covering PE ramp + input-DMA latency

_NC = None


def _build_nc():
    from contextlib import ExitStack

    import concourse.bass as bass
    import concourse.mybir as mybir
    import concourse.tile as tile
    from concourse import bacc

    F32 = mybir.dt.float32
    BF16 = mybir.dt.bfloat16
    Alu = mybir.AluOpType

    nc = bacc.Bacc()
    xt_d = nc.dram_tensor("xt", [DIN, N], BF16, kind="ExternalInput")
    adjt_d = nc.dram_tensor("adjt", [NNB, P, MB, NBW], BF16, kind="ExternalInput")
    w_d = nc.dram_tensor("weight", [DIN, DOUT], F32, kind="ExternalInput")
    b_d = nc.dram_tensor("bias", [DOUT], F32, kind="ExternalInput")
    out_d = nc.dram_tensor("out", [DOUT, N], BF16, kind="ExternalOutput")

    with tile.TileContext(nc) as tc, ExitStack() as ctx:
        singles = ctx.enter_context(tc.tile_pool(name="singles", bufs=1))
        aq_pool = ctx.enter_context(tc.tile_pool(name="aq", bufs=6))
        ot_pool = ctx.enter_context(tc.tile_pool(name="ot", bufs=8))
        psum_s1 = ctx.enter_context(tc.tile_pool(name="ps1", bufs=4, space="PSUM"))
        psum_s2 = ctx.enter_context(tc.tile_pool(name="ps2", bufs=4, space="PSUM"))

        # ---- inputs in flight ------------------------------------------
        # weight on the ACT queue: it is idle this early, and the HWDGE path
        # starts the transfer ~1us sooner than SWDGE descriptor prep would,
        # which matters because dw gates every stage-1 matmul.
        w_sb = singles.tile([P, IB, DOUT], F32)
        nc.scalar.dma_start(out=w_sb, in_=w_d[:].rearrange("(c p) o -> p c o", p=P))
        bias_sb = singles.tile([P, OH], F32)
        nc.scalar.dma_start(out=bias_sb, in_=b_d[:].rearrange("(c p) -> p c", p=P))

        # xt in XW column windows so stage 1 starts on window 0.  The
        # windows are interleaved with nb0's adjT quarters on the SP queue
        # (see below) so stage 2's first block can start right behind
        # stage 1's first chunks.
        xt_sb = singles.tile([P, IB, N], BF16)
        xt_r = xt_d[:].rearrange("(c p) m -> p c m", p=P)
        WN = N // XW

        def start_xt(w):
            nc.sync.dma_start(
                out=xt_sb[:, :, w * WN : (w + 1) * WN],
                in_=xt_r[:, :, w * WN : (w + 1) * WN],
            )

        # ---- PE warm-up ------------------------------------------------
        # The cost model's p-state ramp needs ~3us of continuous PE
        # activity to reach full clock; the real work is gated on the
        # xt/weight DMAs anyway, so spend the wait ramping.
        junk = singles.tile([P, NBW], BF16)
        nc.gpsimd.memset(junk, 1.0)  # Pool engine: keep DVE free for quantize
        jrelu = singles.tile([P, 8], BF16)
        nc.scalar.activation(jrelu, junk[:, 0:8], mybir.ActivationFunctionType.Relu)
        for i in range(WARMUP):
            jp = psum_s2.tile([P, NBW], F32, tag="ps2")
            nc.tensor.matmul(jp, lhsT=junk[:, 0:P], rhs=junk, start=True, stop=True)

        # ---- ternary-quantized weight: dw = ((w > s) - (w < -s)) * s ---
        # Quantized per i-chunk so stage 1's first matmul (which only needs
        # chunk 0) starts before chunk 1's quantize chain finishes.
        dw = singles.tile([P, IB, DOUT], BF16)
        tpos = singles.tile([P, IB, DOUT], F32)
        tneg = singles.tile([P, IB, DOUT], F32)
        for ic in range(IB):
            nc.vector.tensor_scalar(
                out=tpos[:, ic, :], in0=w_sb[:, ic, :],
                scalar1=SPARSITY, scalar2=SPARSITY,
                op0=Alu.is_gt, op1=Alu.mult,
            )
            nc.vector.tensor_scalar(
                out=tneg[:, ic, :], in0=w_sb[:, ic, :],
                scalar1=-SPARSITY, scalar2=SPARSITY,
                op0=Alu.is_lt, op1=Alu.mult,
            )
            nc.vector.tensor_sub(dw[:, ic, :], tpos[:, ic, :], tneg[:, ic, :])

        # ---- stage-2 adjT prefetch stream ------------------------------
        aq_tiles = {}

        def start_aq(nb, q):
            aq = aq_pool.tile([P, QW, NBW], BF16, tag="aq", name=f"aq{nb}_{q}")
            nc.sync.dma_start(out=aq, in_=adjt_d[nb, :, q * QW : (q + 1) * QW, :])
            aq_tiles[(nb, q)] = aq

        # SP-queue DMA order: xt window w, then nb0's quarter q=w.  Stage 1
        # chunk group w and stage 2 (nb0, q=w) consume them in the same
        # rhythm, so the PE starts nb0 right behind stage 1's first chunks
        # instead of after all of stage 1.
        for w in range(XW):
            start_xt(w)
            start_aq(0, w)

        # ---- fused stage 1 + stage 2 -----------------------------------
        # stage 1: support[m-chunk][p, o] = sum_i x[., i] dw[i, o]
        # stage 2: outT[oh, nb*512+n] = relu(sum_m support[m, oh*] adjT + b)
        support = singles.tile([P, MB, DOUT], BF16)

        def s1_chunk(mc):
            sp = psum_s1.tile([P, DOUT], F32, tag="ps1")
            for ic in range(IB):
                nc.tensor.matmul(
                    sp,
                    lhsT=xt_sb[:, ic, mc * P : (mc + 1) * P],
                    rhs=dw[:, ic, :],
                    start=(ic == 0),
                    stop=(ic == IB - 1),
                )
            if mc % 2 == 0:
                nc.vector.tensor_copy(support[:, mc, :], sp)
            else:
                nc.scalar.copy(support[:, mc, :], sp)

        s2_psums = {}

        def s2_quarter(nb, q):
            """Matmuls for stage-2 block nb over m-chunks q*QW..q*QW+3."""
            if q == 0:
                s2_psums[nb] = [
                    psum_s2.tile([P, NBW], F32, tag="ps2", name=f"po{nb}_{oh}")
                    for oh in range(OH)
                ]
            po = s2_psums[nb]
            aq = aq_tiles[(nb, q)]
            for k in range(QW):
                mc = q * QW + k
                rhs = aq[:, k, :]
                for oh in range(OH):
                    nc.tensor.matmul(
                        po[oh],
                        lhsT=support[:, mc, oh * P : (oh + 1) * P],
                        rhs=rhs,
                        start=(mc == 0),
                        stop=(mc == MB - 1),
                    )

        def s2_close(nb):
            """bias+relu evictions in parallel (oh0 on ACT, oh1 on DVE),
            stores on the ACT queue (the only other HWDGE queue is SP,
            which is owned by the adjT prefetch stream)."""
            po = s2_psums[nb]
            ot0 = ot_pool.tile([P, NBW], BF16, tag="ot")
            nc.scalar.activation(
                ot0, po[0], mybir.ActivationFunctionType.Relu,
                bias=bias_sb[:, 0:1],
            )
            ot1 = ot_pool.tile([P, NBW], BF16, tag="ot")
            nc.vector.tensor_scalar(
                out=ot1, in0=po[1], scalar1=bias_sb[:, 1:2], scalar2=0.0,
                op0=Alu.add, op1=Alu.max,
            )
            nc.scalar.dma_start(
                out=out_d[0:P, nb * NBW : (nb + 1) * NBW], in_=ot0
            )
            nc.scalar.dma_start(
                out=out_d[P : 2 * P, nb * NBW : (nb + 1) * NBW], in_=ot1
            )

        # nb0 rides along with stage 1, quarter by quarter; nb1's quarters
        # are prefetched as nb0's are consumed.
        for q in range(NQ):
            for k in range(QW):
                s1_chunk(q * QW + k)
            s2_quarter(0, q)
            start_aq(1, q)
        s2_close(0)

        # remaining blocks: steady-state stream, one block of lookahead.
        for nb in range(1, NNB):
            for q in range(NQ):
                if nb + 1 < NNB:
                    start_aq(nb + 1, q)
                s2_quarter(nb, q)
            s2_close(nb)

    nc.compile()
    return nc


def _get_nc():
    global _NC
    if _NC is None:
        _NC = _build_nc()
    return _NC


def kernel(x, adj, weight, bias, _trace=False):
    import ml_dtypes
    from concourse import bass_utils

    bf16 = ml_dtypes.bfloat16
    x = np.asarray(x, dtype=np.float32)
    adj = np.asarray(adj, dtype=np.float32)
    weight = np.ascontiguousarray(np.asarray(weight, dtype=np.float32))
    bias = np.ascontiguousarray(np.asarray(bias, dtype=np.float32))

    nc = _get_nc()
    in_maps = []
    for b in range(B):
        xt = np.ascontiguousarray(x[b].T).astype(bf16)
        # adjt[nb, p, mc, j] = adj[b][nb*512 + j, mc*128 + p]
        adjt = np.ascontiguousarray(
            adj[b].reshape(NNB, NBW, MB, P).transpose(0, 3, 2, 1)
        ).astype(bf16)
        in_maps.append({"xt": xt, "adjt": adjt, "weight": weight, "bias": bias})

    res = bass_utils.run_bass_kernel_spmd(
        nc, in_maps, core_ids=list(range(B)), trace=_trace
    )
    out = np.stack(
        [np.asarray(r["out"]).astype(np.float32).T for r in res.results], axis=0
    )
    if _trace:
        return out, res
    return out
